# revision 15
# baseline (speedup 1.0000x reference)
"""GAT layer (PyG GATConv H=4,C=64 + PReLU) on 8 Trainium2 NeuronCores — v2.

Strategy (dst-sharded nodes, sharded phase-1 + hardware AllGather):
  - Host: add self loops, sort edges by dst, partition dst nodes across 8
    cores (6250 each) in canonical order (no rotation), group each core's
    edges into 128-dst blocks, tile each block's edges into 128-edge tiles
    with a uniform cross-core tile structure (one SPMD program).
  - Phase 1 (sharded): core m computes h/a_src/a_dst ONLY for its own 6250
    nodes (1/8 of the replicated-phase-1 matmul + x traffic), packs
    [h(256) | a_src(4) | pad] into 768-B rows of a collective input
    buffer, and AllGathers the full 50000-row table into Shared DRAM on
    the TOPSP/SDMA collective hardware (free overlap with the engines).
    a_dst for the core's own 6250 dst nodes stays resident in SBUF.
  - Phase 2: per 128-edge tile, dma_gather table rows by src (two int16
    windows, lo/hi, round-robin across SWDGE queues); one-hot
    S1[e,slot] = (rel_dst[e] == slot) AND its transpose S1T[slot,e]
    (from a host-transposed broadcast copy of rel) are built by
    iota-compare on DVE; per-edge a_dst = S1T^T-matmul against the
    in-SBUF a_dst block (no a_dst gather); p = exp(leaky_relu(a_src +
    a_dst)); messages h*p scatter-added into the block's 128 dst slots by
    matmul (lhsT=S1, rhs=[h*p | p]) accumulating [256 msg | 4 denom] in
    PSUM across the block's tiles.  Epilogue: out = prelu(msg/denom+bias).
  - Padded edges carry rel_dst=200: no iota match -> all-zero one-hot
    column -> zero contribution; no sentinel rows needed.
  - Softmax max-subtraction is skipped (logits are O(1); exp can't
    overflow) making the edge pass single-sweep: out = (Σ p·h)/(Σ p).
"""

import sys

sys.path.insert(0, "/opt/trn_rl_repo")

import numpy as np
import ml_dtypes

import concourse.bass as bass
import concourse.bacc as bacc
import concourse.tile as tile
from concourse import mybir
from concourse.bass import AP

F32 = mybir.dt.float32
BF16 = mybir.dt.bfloat16
I16 = mybir.dt.int16
AF = mybir.ActivationFunctionType
OP = mybir.AluOpType
BF16NP = ml_dtypes.bfloat16

P = 128
NEG_SLOPE = 0.2
REL_PAD = 200.0


class Cfg:
    def __init__(self, n_nodes=50000, in_ch=512, hid=64, heads=4, n_cores=8,
                 tc_max=8, win=32768, n_queues=4, dma_scratch=16384):
        assert n_nodes % n_cores == 0
        assert in_ch % P == 0
        self.n_nodes = n_nodes
        self.in_ch = in_ch
        self.hid = hid
        self.heads = heads
        self.hc = hid * heads                      # 256
        self.rowp = self.hc + 2 * heads            # 264 matmul cols
        self.row = 384                             # table row cols (768 B)
        self.n_cores = n_cores
        self.ndst = n_nodes // n_cores             # 6250
        self.nblk = -(-self.ndst // P)             # 49
        self.kt = in_ch // P                       # 4
        self.tc_max = tc_max
        self.win = min(win, n_nodes)
        assert n_nodes <= 2 * self.win, "lo+hi windows must cover table"
        self.n_queues = n_queues
        self.dma_scratch = dma_scratch


CFG = Cfg()


def _wrap16(flat):
    """int16 index list -> dma_gather layout [128, n/16]."""
    n = len(flat)
    assert n % 16 == 0
    a = np.asarray(flat, dtype=np.int16).reshape(n // 16, 16).T  # [16, n/16]
    return np.tile(a, (8, 1))                                    # [128, n/16]


# ---------------------------------------------------------------- host prep

def host_prep_edges(edge_index, cfg):
    """Sort/partition/tile edges.  Returns (per_core data dicts, meta)."""
    n = cfg.n_nodes
    loop = np.arange(n, dtype=np.int64)
    src = np.concatenate([edge_index[0].astype(np.int64), loop])
    dst = np.concatenate([edge_index[1].astype(np.int64), loop])
    order = np.argsort(dst, kind="stable")
    src_s = src[order]
    dst_s = dst[order]

    lo_rows = cfg.win                 # lo window = rows [0, win)
    hi_base = n - cfg.win             # hi window = rows [hi_base, n)

    # per-(core, block) segments; lo/hi split
    seg = {}
    tl_req = np.zeros((cfg.n_cores, cfg.nblk), dtype=np.int64)
    th_req = np.zeros((cfg.n_cores, cfg.nblk), dtype=np.int64)
    for m in range(cfg.n_cores):
        base = m * cfg.ndst
        for b in range(cfg.nblk):
            d0 = base + b * P
            d1 = min(base + (b + 1) * P, base + cfg.ndst)
            lo = np.searchsorted(dst_s, d0)
            hi = np.searchsorted(dst_s, d1)
            s_e = src_s[lo:hi]
            d_loc = dst_s[lo:hi] - d0                 # slot in block
            is_lo = s_e < lo_rows
            seg[(m, b)] = (s_e, d_loc, is_lo)
            tl_req[m, b] = -(-int(is_lo.sum()) // P)
            th_req[m, b] = -(-int((~is_lo).sum()) // P)
    tl = tl_req.max(axis=0).astype(int)
    th = th_req.max(axis=0).astype(int)
    for b in range(cfg.nblk):
        if tl[b] + th[b] == 0:
            tl[b] = 1

    # uniform chunk structure: per block, lo tiles then hi tiles,
    # split at tc_max
    chunks = []          # (block, t0_global, ntiles, nidx, icol0, half)
    tiles_b = tl + th
    col0 = np.concatenate([[0], np.cumsum(tiles_b)])[:-1].astype(int)
    T = int(tiles_b.sum())
    icol = 0
    qn = 0
    for b in range(cfg.nblk):
        t0 = int(col0[b])
        for half, nt_half in (("lo", int(tl[b])), ("hi", int(th[b]))):
            q0 = 0
            while q0 < nt_half:
                qq = min(cfg.tc_max, nt_half - q0)
                chunks.append(dict(b=b, t0=t0 + q0, nt=qq, nidx=qq * P,
                                   icol=icol, half=half, qn=qn))
                qn += 1
                icol += qq * P // 16
                q0 += qq
            t0 += nt_half
    icol_main = icol

    per_core = []
    for m in range(cfg.n_cores):
        rel_all = np.full((P, T), REL_PAD, dtype=np.float32)
        midx_all = np.zeros((P, icol_main), dtype=np.int16)
        tbuf = {}
        for b in range(cfg.nblk):
            s_e, d_loc, is_lo = seg[(m, b)]
            for half, nt_half in (("lo", int(tl[b])), ("hi", int(th[b]))):
                sel = is_lo if half == "lo" else ~is_lo
                ne = int(sel.sum())
                npad = nt_half * P
                bs = np.full(npad, 0 if half == "lo" else hi_base,
                             dtype=np.int64)
                br = np.full(npad, int(REL_PAD), dtype=np.int64)
                bs[:ne] = s_e[sel]
                br[:ne] = d_loc[sel]
                tbuf[(b, half)] = (bs, br)
        for b in range(cfg.nblk):
            t0 = int(col0[b])
            for half, nt_half in (("lo", int(tl[b])), ("hi", int(th[b]))):
                if nt_half == 0:
                    continue
                bs, br = tbuf[(b, half)]
                rel_all[:, t0:t0 + nt_half] = br.reshape(nt_half, P).T
                t0 += nt_half
        for ch in chunks:
            b = ch["b"]
            half = ch["half"]
            bs, _ = tbuf[(b, half)]
            base_t = int(col0[b]) + (int(tl[b]) if half == "hi" else 0)
            off = (ch["t0"] - base_t) * P
            flat = bs[off:off + ch["nidx"]].copy()
            if half == "hi":
                flat -= hi_base
            assert flat.min() >= 0 and flat.max() < cfg.win, (
                flat.min(), flat.max(), half)
            midx_all[:, ch["icol"]:ch["icol"] + ch["nidx"] // 16] = \
                _wrap16(flat)
        rel2 = np.repeat(rel_all.astype(BF16NP), 2, axis=1)   # [P, 2T]
        relT = np.ascontiguousarray(rel_all.T.astype(BF16NP))  # [T, P]
        per_core.append(dict(
            midx=np.ascontiguousarray(midx_all),
            rel=np.ascontiguousarray(rel2).view(np.int16),
            relT=relT,
        ))
    meta = dict(chunks=chunks, tiles_b=tiles_b, col0=col0, T=T,
                icol_main=icol_main, hi_base=hi_base)
    return per_core, meta


def host_prep_weights(x, lin_w, att_src, att_dst, bias, prelu_w, cfg):
    n, ic, h, c = cfg.n_nodes, cfg.in_ch, cfg.heads, cfg.hid
    w3 = lin_w.astype(np.float64).reshape(h, c, ic)
    wa_src = (w3 * att_src.astype(np.float64).reshape(h, c, 1)).sum(1).T
    wa_dst = (w3 * att_dst.astype(np.float64).reshape(h, c, 1)).sum(1).T
    lwT = lin_w.astype(np.float64).T                           # [ic, 256]
    lwTi = lwT.reshape(ic, h, c).transpose(0, 2, 1).reshape(ic, h * c)
    rhs = np.concatenate([wa_dst, lwTi, wa_src], axis=1)       # [ic, 264]
    rhs_w = rhs.astype(BF16NP)
    # partition-packed weight block: wp[p, k*264 + r] = rhs_w[k*128 + p, r]
    wp = np.ascontiguousarray(
        rhs_w.reshape(cfg.kt, P, cfg.rowp).transpose(1, 0, 2)
        .reshape(P, cfg.kt * cfg.rowp)).view(np.int16)
    def inter(v):
        return v.reshape(h, c).T.reshape(h * c)
    bias_rep = np.ascontiguousarray(np.broadcast_to(
        inter(bias.astype(np.float32)).astype(BF16NP),
        (P, cfg.hc))).view(np.int16)
    pw_rep = np.ascontiguousarray(np.broadcast_to(
        inter(prelu_w.astype(np.float32)).astype(BF16NP),
        (P, cfg.hc))).view(np.int16)
    xbf = x.astype(BF16NP)
    # partition-packed x block: xp[p, k*ndst + q] = x[base+q, :].T[k*128+p]
    xps = []
    for m in range(cfg.n_cores):
        xTm = xbf[m * cfg.ndst:(m + 1) * cfg.ndst].T          # [512, ndst]
        xps.append(np.ascontiguousarray(
            xTm.reshape(cfg.kt, P, cfg.ndst).transpose(1, 0, 2)
            .reshape(P, cfg.kt * cfg.ndst)).view(np.int16))
    return dict(wp=wp, bias_rep=bias_rep, pw_rep=pw_rep, xps=xps)


# ---------------------------------------------------------------- builder

def build(cfg, meta, bias_nonzero=True, parts=None, act_lrelu=False):
    parts = parts or {"p1", "cc", "gather", "s1", "adst", "pcomp", "mm",
                      "epi"}
    n, row, hc, h = cfg.n_nodes, cfg.row, cfg.hc, cfg.heads
    nc = bacc.Bacc(num_devices=cfg.n_cores, num_swdge_queues=cfg.n_queues,
                   enable_partition_id=False,
                   dynamic_dma_scratch_size=cfg.dma_scratch)

    T = meta["T"]
    # packed 16-bit parameter layout (per partition-row):
    #   [midx | rel(bf16) | bias(bf16) | pw(bf16) | w(bf16) | x(bf16)]
    c_rel = meta["icol_main"]
    c_bias = c_rel + 2 * T
    c_pw = c_bias + hc
    c_w = c_pw + hc
    c_x = c_w + cfg.kt * cfg.rowp
    sb_cols = c_x + cfg.kt * cfg.ndst
    meta["pk_sb_cols"] = sb_cols

    packed = nc.declare_dram_parameter("packed", [P, sb_cols], I16,
                                       isOutput=False)
    out = nc.declare_dram_parameter("out", [cfg.ndst, hc], F32, isOutput=True)

    cin = nc.dram_tensor("cin", [cfg.ndst, row], BF16)
    table = nc.dram_tensor("table", [n, row], BF16, addr_space="Shared")
    hi_base = meta["hi_base"]

    with tile.TileContext(nc) as tc:
        fpool_cm = tc.tile_pool(name="fp", bufs=1)
        fpool = fpool_cm.__enter__()

        pk = fpool.tile([P, sb_cols], I16)
        nc.sync.dma_start(out=pk[:], in_=packed[:, :])
        pkv = pk[:]

        def pk_view(col0, dims, dtype=None):
            v = AP(pkv.tensor, pkv.offset + col0, [pkv.ap[0]] + dims)
            return v.bitcast(dtype) if dtype is not None else v

        bias_sb = fpool.tile([P, hc], F32)
        nc.vector.tensor_copy(out=bias_sb[:],
                              in_=pk_view(c_bias, [[1, hc]], BF16))
        pw_sb = fpool.tile([P, hc], F32)
        nc.vector.tensor_copy(out=pw_sb[:],
                              in_=pk_view(c_pw, [[1, hc]], BF16))

        # iota_bf[p, j] = j ; chan_bf[p, j] = p   (both [128, 128] bf16)
        iota_i = fpool.tile([P, P], mybir.dt.int32)
        nc.gpsimd.iota(iota_i[:], pattern=[[1, P]], base=0,
                       channel_multiplier=0)
        iota_f = fpool.tile([P, P], F32)
        nc.vector.tensor_copy(out=iota_f[:], in_=iota_i[:])
        iota_bf = fpool.tile([P, P], BF16)
        nc.vector.tensor_copy(out=iota_bf[:], in_=iota_f[:])
        chan_i = fpool.tile([P, P], mybir.dt.int32)
        nc.gpsimd.iota(chan_i[:], pattern=[[0, P]], base=0,
                       channel_multiplier=1)
        chan_f = fpool.tile([P, P], F32)
        nc.vector.tensor_copy(out=chan_f[:], in_=chan_i[:])
        chan_bf = fpool.tile([P, P], BF16)
        nc.vector.tensor_copy(out=chan_bf[:], in_=chan_f[:])
        ident_bf = fpool.tile([P, P], BF16)
        nc.vector.tensor_tensor(out=ident_bf[:], in0=iota_bf[:],
                                in1=chan_bf[:], op=OP.is_equal)

        # a_dst for this core's own dst nodes, by (slot, block, head);
        # zero-init so the partial last block's unused slots are benign
        adst_sb = fpool.tile([P, cfg.nblk, h], BF16)
        nc.vector.memset(adst_sb[:], 0.0)

        # ---------------- phase 1: sharded table build ----------------
        with (
            tc.tile_pool(name="p1o", bufs=4) as opool,
            tc.tile_pool(name="p1ps", bufs=4, space="PSUM") as pspool,
        ):
            if "p1" in parts:
                for t in range(cfg.nblk):
                    t0 = t * P
                    mm = min(P, cfg.ndst - t0)
                    ps = pspool.tile([P, cfg.rowp], F32, tag="ps")
                    for k in range(cfg.kt):
                        nc.tensor.matmul(
                            ps[:mm, :],
                            lhsT=pk_view(c_x + k * cfg.ndst + t0,
                                         [[1, mm]], BF16),
                            rhs=pk_view(c_w + k * cfg.rowp,
                                        [[1, cfg.rowp]], BF16),
                            start=(k == 0), stop=(k == cfg.kt - 1))
                    hrow = opool.tile([P, row], BF16, tag="hrow")
                    # row = [h(256) | a_src(4) | pad(124)]
                    if mm < P:
                        nc.vector.memset(hrow[:, 0:hc + h], 0.0)
                    nc.scalar.copy(out=hrow[:mm, 0:hc + h],
                                   in_=ps[:mm, h:cfg.rowp])
                    nc.vector.memset(hrow[:, hc + h:row], 0.0)
                    # a_dst block stays in SBUF
                    nc.vector.tensor_copy(out=adst_sb[:mm, t, :],
                                          in_=ps[:mm, 0:h])
                    nc.sync.dma_start(out=cin[t0:t0 + mm, :],
                                      in_=hrow[:mm, :])

            if "cc" in parts:
                nc.gpsimd.collective_compute(
                    "AllGather", mybir.AluOpType.bypass,
                    replica_groups=[list(range(cfg.n_cores))],
                    ins=[cin[:, :]], outs=[table[:, :]])

        # ---------------- phase 2: edge pass ----------------
        with (
            tc.tile_pool(name="p2g", bufs=6) as gpool,
            tc.tile_pool(name="p2s", bufs=6) as s1pool,
            tc.tile_pool(name="p2e", bufs=3) as epool,
            tc.tile_pool(name="p2o", bufs=3) as obpool,
            tc.tile_pool(name="p2ps", bufs=2, space="PSUM") as ps2pool,
            tc.tile_pool(name="p2pt", bufs=2, space="PSUM") as ptpool,
            tc.tile_pool(name="p2tp", bufs=2, space="PSUM") as tpspool,
        ):
            blk_chunks = {}
            for ch in meta["chunks"]:
                blk_chunks.setdefault(ch["b"], []).append(ch)
            for b in range(cfg.nblk):
                chs = blk_chunks[b]
                ps = ps2pool.tile([P, hc + h], F32, tag="psb")
                for ci, ch in enumerate(chs):
                    qq = ch["nt"]
                    t0 = ch["t0"]
                    g = gpool.tile([P, qq, row], BF16, tag="g")
                    if "gather" in parts:
                        if ch["half"] == "lo":
                            in_ap = table[0:cfg.win, :]
                        else:
                            in_ap = table[hi_base:n, :]
                        nc.gpsimd.dma_gather(
                            out_ap=g[:],
                            in_ap=in_ap,
                            idxs_ap=pk_view(ch["icol"],
                                            [[1, ch["nidx"] // 16]]),
                            num_idxs=ch["nidx"],
                            num_idxs_reg=ch["nidx"],
                            elem_size=row,
                            elem_step=row,
                            queue_num=ch["qn"] % cfg.n_queues)

                    # one-hot S1[e, q, slot] = (rel[e, q] == slot)
                    s1 = s1pool.tile([P, qq, P], BF16, tag="s1")
                    if "s1" in parts:
                        rel_b = AP(pkv.tensor, pkv.offset + c_rel + 2 * t0,
                                   [pkv.ap[0], [2, qq], [0, P // 2],
                                    [1, 2]]).bitcast(BF16)
                        iap = iota_bf[:]
                        iota_b = AP(iap.tensor, iap.offset,
                                    [iap.ap[0], [0, qq], [2, P // 2], [1, 2]])
                        s1v = s1[:]
                        s1_b = AP(s1v.tensor, s1v.offset,
                                  [s1v.ap[0], [P, qq], [2, P // 2], [1, 2]])
                        nc.vector.tensor_tensor(
                            out=s1_b, in0=rel_b, in1=iota_b, op=OP.is_equal)

                    # transposed one-hot S1T = S1^T via PE transpose, then
                    # per-edge a_dst = S1T^T-matmul against the block's
                    # in-SBUF a_dst values
                    adst_ps = ptpool.tile([P, qq * h], F32, tag="adps")
                    if "adst" in parts:
                        s1t = s1pool.tile([P, qq, P], BF16, tag="s1t")
                        for j0 in range(0, qq, 4):
                            jn = min(4, qq - j0)
                            tps = tpspool.tile([P, 4, P], BF16, tag="tps")
                            for j in range(j0, j0 + jn):
                                nc.tensor.transpose(tps[:, j - j0, :],
                                                    s1[:, j, :],
                                                    ident_bf[:])
                            nc.scalar.copy(out=s1t[:, j0:j0 + jn, :],
                                           in_=tps[:, 0:jn, :])
                        for j in range(qq):
                            nc.tensor.matmul(
                                adst_ps[:, j * h:(j + 1) * h],
                                lhsT=s1t[:, j, :],
                                rhs=adst_sb[:, b, :],
                                start=True, stop=True)

                    if "pcomp" in parts:
                        # p = exp(leaky_relu(a_src + a_dst))
                        ef = epool.tile([P, qq, h], F32, tag="ef")
                        adv = adst_ps[:]
                        ad3 = AP(adv.tensor, adv.offset,
                                 [adv.ap[0], [h, qq], [1, h]])
                        nc.vector.tensor_add(
                            out=ef[:],
                            in0=g[:, :, hc:hc + h],
                            in1=ad3)
                        if act_lrelu:
                            nc.scalar.activation(out=ef[:], in_=ef[:],
                                                 func=AF.Lrelu,
                                                 alpha=NEG_SLOPE)
                        else:  # CoreSim has no Lrelu — DVE equivalent
                            ng = epool.tile([P, qq, h], F32, tag="ng")
                            nc.vector.tensor_scalar(
                                out=ng[:], in0=ef[:], scalar1=0.0,
                                scalar2=NEG_SLOPE, op0=OP.min, op1=OP.mult)
                            nc.vector.scalar_tensor_tensor(
                                out=ef[:], in0=ef[:], scalar=0.0,
                                op0=OP.max, in1=ng[:], op1=OP.add)
                        nc.scalar.activation(out=ef[:], in_=ef[:],
                                             func=AF.Exp)
                        nc.vector.tensor_copy(
                            out=g[:, :, hc:hc + h], in_=ef[:])
                        # msg = h * p; h interleaved [c, hh] so the
                        # broadcast AP keeps a stride-1 last dim (2x DVE)
                        msg = g[:, :, 0:hc].rearrange(
                            "p q (c hh) -> p q c hh", hh=h)
                        pslice = g[:, :, hc:hc + h]
                        pb = AP(pslice.tensor, pslice.offset,
                                [pslice.ap[0], pslice.ap[1],
                                 [0, cfg.hid], [1, h]])
                        nc.vector.tensor_tensor(out=msg, in0=msg, in1=pb,
                                                op=OP.mult)

                    if "mm" in parts:
                        for j in range(qq):
                            nc.tensor.matmul(
                                ps[:, :],
                                lhsT=s1[:, j, :],
                                rhs=g[:, j, 0:hc + h],
                                start=(ci == 0 and j == 0),
                                stop=(ci == len(chs) - 1 and j == qq - 1))

                # epilogue: out = prelu(msg/denom + bias)
                if "epi" not in parts or "mm" not in parts:
                    continue
                den = epool.tile([P, h], F32, tag="den")
                nc.vector.tensor_scalar_add(out=den[:], in0=ps[:, hc:hc + h],
                                            scalar1=1e-6)
                rec = epool.tile([P, h], F32, tag="rec")
                nc.vector.reciprocal(out=rec[:], in_=den[:])
                ob = obpool.tile([P, hc], BF16, tag="ob")
                recb = AP(rec.tensor, rec[:].offset,
                          [rec[:].ap[0], [0, cfg.hid], [1, h]])
                nc.vector.tensor_tensor(
                    out=ob[:].rearrange("p (c hh) -> p c hh", hh=h),
                    in0=ps[:, 0:hc].rearrange("p (c hh) -> p c hh", hh=h),
                    in1=recb, op=OP.mult)
                if bias_nonzero:
                    nc.vector.tensor_add(out=ob[:], in0=ob[:], in1=bias_sb[:])
                t2 = obpool.tile([P, hc], BF16, tag="t2")
                nc.vector.scalar_tensor_tensor(
                    out=t2[:], in0=ob[:], scalar=0.0, op0=OP.min,
                    in1=pw_sb[:], op1=OP.mult)
                obp = obpool.tile([P, hc], F32, tag="obp")
                obpv = obp[:]
                # write through a permuted view: col c*4+hh -> hh*64+c
                obp_perm = AP(obpv.tensor, obpv.offset,
                              [obpv.ap[0], [cfg.hid, h], [1, cfg.hid]])
                iview = [[1, h], [h, cfg.hid]]
                ob_i = AP(ob[:].tensor, ob[:].offset, [ob[:].ap[0]] + iview)
                t2_i = AP(t2[:].tensor, t2[:].offset, [t2[:].ap[0]] + iview)
                nc.vector.scalar_tensor_tensor(
                    out=obp_perm, in0=ob_i, scalar=0.0, op0=OP.max,
                    in1=t2_i, op1=OP.add)
                rows = min(P, cfg.ndst - b * P)
                nc.sync.dma_start(out=out[b * P:b * P + rows, :],
                                  in_=obp[:rows, :])
        fpool_cm.__exit__(None, None, None)
    return nc


# ---------------------------------------------------------------- runner

def _prepare(x, edge_index, lin_w, att_src, att_dst, bias, prelu_w, cfg):
    per_core, meta = host_prep_edges(np.asarray(edge_index), cfg)
    shared = host_prep_weights(np.asarray(x), np.asarray(lin_w),
                               np.asarray(att_src), np.asarray(att_dst),
                               np.asarray(bias), np.asarray(prelu_w), cfg)
    bias_nonzero = bool(np.any(np.asarray(bias) != 0))
    nc = build(cfg, meta, bias_nonzero=bias_nonzero)
    in_maps = []
    for m in range(cfg.n_cores):
        packed = np.concatenate(
            [per_core[m]["midx"], per_core[m]["rel"], shared["bias_rep"],
             shared["pw_rep"], shared["wp"], shared["xps"][m]], axis=1)
        assert packed.shape == (P, meta["pk_sb_cols"])
        assert packed.dtype == np.int16
        in_maps.append(dict(packed=np.ascontiguousarray(packed)))
    return nc, in_maps


def _run_pjrt(nc, in_maps, n_cores, time_iters=0):
    """Compile+run via PJRT (axon); optionally time warm re-executions."""
    import time
    import jax
    from jax.sharding import Mesh, PartitionSpec
    from jax.experimental.shard_map import shard_map
    from concourse import bass2jax, mybir as mb

    bass2jax.install_neuronx_cc_hook()
    assert nc.dbg_addr is None
    partition_name = (nc.partition_id_tensor.name
                      if nc.partition_id_tensor else None)
    in_names, out_names, out_avals, zero_outs = [], [], [], []
    for alloc in nc.m.functions[0].allocations:
        if not isinstance(alloc, mb.MemoryLocationSet):
            continue
        name = alloc.memorylocations[0].name
        if alloc.kind == "ExternalInput":
            if name != partition_name:
                in_names.append(name)
        elif alloc.kind == "ExternalOutput":
            shape = tuple(alloc.tensor_shape)
            dtype = mb.dt.np(alloc.dtype)
            out_names.append(name)
            out_avals.append(jax.core.ShapedArray(shape, dtype))
            zero_outs.append(np.zeros(shape, dtype))
    n_params = len(in_names)
    in_names.extend(out_names)
    if partition_name is not None:
        in_names.append(partition_name)

    def _body(*args):
        operands = list(args)
        if partition_name is not None:
            operands.append(bass2jax.partition_id_tensor())
        outs = bass2jax._bass_exec_p.bind(
            *operands,
            out_avals=tuple(out_avals),
            in_names=tuple(in_names),
            out_names=tuple(out_names),
            lowering_input_output_aliases=(),
            sim_require_finite=True,
            sim_require_nnan=True,
            nc=nc,
        )
        return tuple(outs)

    devices = jax.devices()[:n_cores]
    mesh = Mesh(np.asarray(devices), ("core",))
    n_outs = len(out_avals)
    in_specs = (PartitionSpec("core"),) * (n_params + n_outs)
    out_specs = (PartitionSpec("core"),) * n_outs
    sharded = jax.jit(
        shard_map(_body, mesh=mesh, in_specs=in_specs, out_specs=out_specs,
                  check_rep=False),
        keep_unused=True,
    )
    per_core = [[np.asarray(m[name]) for name in in_names[:n_params]]
                for m in in_maps]
    concat_in = [
        np.concatenate([per_core[c][i] for c in range(n_cores)], axis=0)
        for i in range(n_params)
    ]
    concat_zeros = [
        np.zeros((n_cores * z.shape[0], *z.shape[1:]), z.dtype)
        for z in zero_outs
    ]
    sh = jax.sharding.NamedSharding(mesh, PartitionSpec("core"))
    dev_args = [jax.device_put(a, sh) for a in concat_in + concat_zeros]
    out_arrs = sharded(*dev_args)
    jax.block_until_ready(out_arrs)
    t_ns = None
    if time_iters > 0:
        # Back-to-back dispatches pipeline on the device queue: time M
        # executions blocking once, for two M values, and use the slope.
        def loop_wall(mreps):
            best = None
            for _ in range(time_iters):
                t0 = time.perf_counter_ns()
                o = None
                for _ in range(mreps):
                    o = sharded(*dev_args)
                jax.block_until_ready(o)
                dt = time.perf_counter_ns() - t0
                best = dt if best is None else min(best, dt)
            return best

        m1, m2 = 20, 120
        w1 = loop_wall(m1)
        w2 = loop_wall(m2)
        t_ns = max(0, (w2 - w1) // (m2 - m1))
    results = [
        {name: np.asarray(out_arrs[i]).reshape(n_cores, *out_avals[i].shape)[c]
         for i, name in enumerate(out_names)}
        for c in range(n_cores)
    ]
    return results, t_ns


def run(x, edge_index, lin_w, att_src, att_dst, bias, prelu_w,
        cfg=None, time_iters=0):
    cfg = cfg or CFG
    nc, in_maps = _prepare(x, edge_index, lin_w, att_src, att_dst, bias,
                           prelu_w, cfg)
    nc.finalize()
    results, t_ns = _run_pjrt(nc, in_maps, cfg.n_cores,
                              time_iters=time_iters)
    outs = [results[m]["out"] for m in range(cfg.n_cores)]
    full = np.concatenate(outs, axis=0).astype(np.float32)
    return full, t_ns


def kernel(**inputs):
    full, _ = run(inputs["x"], inputs["edge_index"], inputs["lin_w"],
                  inputs["att_src"], inputs["att_dst"], inputs["bias"],
                  inputs["prelu_w"])
    return full
